# revision 25
# baseline (speedup 1.0000x reference)
"""GQA causal attention (S=2048, H=32, KVH=8, D=128) on 8 TRN2 NeuronCores.

Sharding: tensor-parallel over heads. Core i computes query heads
[4i, 4i+4) against KV head i (GQA group size 32/8 = 4). No collectives:
the host slices the inputs per core and concatenates the outputs.

Per-core algorithm (seq=2048, d=128, 4 q-heads, 1 kv-head, causal):
  - K^T and per-head Q^T staged in SBUF as [d=128, seq] bf16
    (PE transposes via identity matmul; fp32 DMA-transpose is unsupported
    and the xbar ucode transpose costs ~1.3us of SP-engine time per call).
  - V staged naturally as [128, 16, 129] bf16 tiles with a ones column
    appended, so the PV matmul also produces the softmax denominator.
  - For each head, for each key-tile kt (128 keys):
      S^T[kt]  = (K^T tile).T @ Q^T          -> PSUM [128, qspan] fp32,
                 exact-causal: only q >= kt*128 is computed
      P^T[kt]  = exp(SCALE * S^T[kt])        -> SBUF bf16 (wide ACTIVATEs;
                 scores are O(1) so no max-subtraction is needed)
      the diagonal 128-column block is masked with gpsimd.affine_select
  - For each query-tile qt: acc[qt] = sum_kt (P^T tile).T @ [V | 1]
      accumulated in PSUM over kt; column 128 is the denominator.
      DVE reciprocal + tensor_scalar_mul normalizes into a per-q-block
      staging buffer; one batched DMA per 512 rows stores the output.
"""

import numpy as np

SEQ = 2048
D = 128
QH = 4  # query heads per core
N_CORES = 8
SCALE = 0.08838834764831845  # 1/sqrt(128)
NT = SEQ // 128  # 16 tiles of 128 along seq

_NC = None


def _emit(ctx, tc, q, k, v, out):
    import concourse.mybir as mybir
    from concourse import masks

    nc = tc.nc
    f32 = mybir.dt.float32
    bf16 = mybir.dt.bfloat16
    Exp = mybir.ActivationFunctionType.Exp

    # Every DMA destination gets a dedicated (never-recycled) buffer: a
    # reused slot would add extra semaphore waits on the HWDGE DMA.
    singles = ctx.enter_context(tc.tile_pool(name="singles", bufs=1))
    qpool = ctx.enter_context(tc.tile_pool(name="qpool", bufs=2))
    ppool = ctx.enter_context(tc.tile_pool(name="ppool", bufs=2))
    opool = ctx.enter_context(tc.tile_pool(name="opool", bufs=3))
    # PSUM budget (8 banks): scores 2x2 + out-acc 2x1 + transposes 2x1.
    psum_s = ctx.enter_context(tc.tile_pool(name="psum_s", bufs=2, space="PSUM"))
    psum_o = ctx.enter_context(tc.tile_pool(name="psum_o", bufs=2, space="PSUM"))
    psum_t = ctx.enter_context(tc.tile_pool(name="psum_t", bufs=2, space="PSUM"))

    # ---- PE warmup: dense dummy matmuls while the DMA prep runs, so the
    # HAM clock-gate reaches 2.4 GHz by the time real PE work arrives.
    warm_src = singles.tile([128, 512], bf16, tag="warm_src")
    nc.gpsimd.memset(warm_src[:], 0.0)
    warm_ps = psum_o.tile([128, 512], f32, tag="o")
    for _ in range(12):
        nc.tensor.matmul(
            warm_ps[:], lhsT=warm_src[:, 0:128], rhs=warm_src[:], start=True, stop=True
        )

    ident = singles.tile([128, 128], bf16)
    masks.make_identity(nc, ident[:])
    keep = singles.tile([128, 128], bf16)
    masks.make_upper_triangular(nc, keep[:], val=1.0, diag=True)

    kT = singles.tile([128, SEQ], bf16)
    knat = singles.tile([128, NT, 128], f32, tag="knat")
    knat_bf = singles.tile([128, NT, 128], bf16, tag="knat_bf")
    kr = k.rearrange("(t p) d -> p t d", p=128)

    def kchunk(c, copy_eng):
        """Load + cast + PE-transpose one 4-tile chunk of K into kT."""
        cs = slice(c * 4, (c + 1) * 4)
        nc.sync.dma_start(out=knat[:, cs, :], in_=kr[:, cs, :])
        nc.vector.tensor_copy(knat_bf[:, cs, :], knat[:, cs, :])
        for t in range(c * 4, (c + 1) * 4):
            pst = psum_t.tile([128, 128], bf16, tag="tp")
            nc.tensor.transpose(pst[:], knat_bf[:, t, :], ident[:])
            copy_eng(kT[:, t * 128:(t + 1) * 128], pst[:])

    def qprep_alloc(h):
        qnat = singles.tile([128, NT, 128], f32, tag=f"qnat{h}")
        qnat_bf = singles.tile([128, NT, 128], bf16, tag=f"qnat_bf{h}")
        qT = qpool.tile([128, SEQ], bf16, tag="qT")
        return qnat, qnat_bf, qT

    def qprep_chunk(h, st, c):
        """Load + cast + PE-transpose one 4-tile chunk of head h's Q."""
        qnat, qnat_bf, qT = st
        qrh = q[:, h * D:(h + 1) * D].rearrange("(t p) d -> p t d", p=128)
        cs = slice(c * 4, (c + 1) * 4)
        nc.sync.dma_start(out=qnat[:, cs, :], in_=qrh[:, cs, :])
        nc.vector.tensor_copy(qnat_bf[:, cs, :], qnat[:, cs, :])
        for t in range(c * 4, (c + 1) * 4):
            pst = psum_t.tile([128, 128], bf16, tag="tp")
            nc.tensor.transpose(pst[:], qnat_bf[:, t, :], ident[:])
            nc.vector.tensor_copy(qT[:, t * 128:(t + 1) * 128], pst[:])

    def emit_qprep(h):
        st = qprep_alloc(h)
        for c in range(4):
            qprep_chunk(h, st, c)
        return st[2]

    # ---- Prep, ordered for shortest path to the first QK matmul: K chunk 0
    # and head-0 Q chunks 0-1 only; the rest is emitted inside the head-0
    # kt loop so the PE's in-order stream reaches QK(kt=0) early.
    kchunk(0, nc.vector.tensor_copy)
    q0st = qprep_alloc(0)
    qprep_chunk(0, q0st, 0)
    qprep_chunk(0, q0st, 1)
    qT = q0st[2]

    # ---- V: natural [128, t, d] bf16 + ones column for the denominator
    vp = singles.tile([128, NT, D + 1], bf16)
    vnat = singles.tile([128, NT, 128], f32, tag="vnat")

    def vprep():
        nc.sync.dma_start(out=vnat[:], in_=v.rearrange("(t p) d -> p t d", p=128))
        nc.vector.tensor_copy(vp[:, :, 0:D], vnat[:])
        nc.vector.memset(vp[:, :, D:D + 1], 1.0)

    def emit_pv(h, qt, pT, vp, osb, ops_tri):
        """O[qt] = sum_k2 pT[k2][:, qt-slice].T @ [V|1], then normalize."""
        ops = ops_tri[:, qt % 3, :]
        for k2 in range(qt + 1):
            nc.tensor.matmul(
                ops,
                lhsT=pT[k2][:, (qt - k2) * 128:(qt - k2) * 128 + 128],
                rhs=vp[:, k2, :],
                start=(k2 == 0),
                stop=(k2 == qt),
            )
        rec = opool.tile([128, 1], f32, tag="rec")
        nc.vector.reciprocal(rec[:], ops[:, D:D + 1])
        nc.vector.tensor_scalar_mul(osb[:, qt % 2, :], ops[:, 0:D], rec[:])
        if qt % 2 == 1:
            qb = qt // 2
            nc.sync.dma_start(
                out=out[qb * 256:(qb + 1) * 256, h * D:(h + 1) * D].rearrange(
                    "(j p) d -> p j d", p=128
                ),
                in_=osb[:],
            )
    def emit_qk_exp(qT, kt, pT_kt, off, cw):
        """One exact-causal S^T chunk ([k0+off, k0+off+cw)) + its exp."""
        k0 = kt * 128
        pw = ((cw + 511) // 512) * 512
        sp = psum_s.tile([128, pw], f32, tag="s")
        m = 0
        while m < cw:
            w = min(512, cw - m)
            nc.tensor.matmul(
                sp[:, m:m + w],
                lhsT=kT[:, k0:k0 + 128],
                rhs=qT[:, k0 + off + m:k0 + off + m + w],
                start=True,
                stop=True,
            )
            m += w
        nc.scalar.activation(pT_kt[:, off:off + cw], sp[:, 0:cw], Exp, scale=SCALE)

    # Pending-PV queue: PV work is emitted two QK steps behind, flowing
    # across head boundaries so neither the PE nor ScalarE sees a bubble
    # between heads.
    pvq = []
    pv_state = {}

    def pop_pv():
        h2, qt2, pT2 = pvq.pop(0)
        st = pv_state.setdefault(h2, {})
        if qt2 % 2 == 0:
            osb = opool.tile([128, 2, D], f32, tag="osb")
            st["osb"] = osb
        if qt2 % 3 == 0:
            ops = psum_o.tile([128, 3, D + 1], f32, tag="o")
            st["ops"] = ops
        emit_pv(h2, qt2, pT2, vp, st["osb"], st["ops"])

    for h in range(QH):
        qT_next = None
        pT = []
        for kt in range(NT):
            k0 = kt * 128
            span = SEQ - k0
            pT_kt = ppool.tile([128, span], bf16, tag=f"pT{kt}")
            # Exact-causal S^T in left-aligned PSUM chunks of <=1024
            # (2 banks), one wide exp each. On head 0's first key-tile the
            # remaining prep is interleaved between chunks so the PE
            # reaches the first QK matmul as early as possible.
            off = 0
            while off < span:
                cw = min(1024, span - off)
                emit_qk_exp(qT, kt, pT_kt, off, cw)
                off += cw
                if h == 0 and kt == 0 and off == 1024:
                    qprep_chunk(0, q0st, 2)
                    qprep_chunk(0, q0st, 3)
            # causal mask on the diagonal 128-col block: keep where q >= k
            nc.vector.tensor_mul(pT_kt[:, 0:128], pT_kt[:, 0:128], keep[:])
            pT.append(pT_kt)
            if h == 0 and kt < 3:
                kchunk(kt + 1, nc.vector.tensor_copy)
                if kt == 0:
                    vprep()
            pvq.append((h, kt, pT))
            while len(pvq) > 3:
                pop_pv()
            # prefetch the next head's Q transposes into the PE stream early
            if kt == 10 and h + 1 < QH:
                qT_next = emit_qprep(h + 1)
        if qT_next is not None:
            qT = qT_next
    while pvq:
        pop_pv()


def _build():
    import concourse.mybir as mybir
    import concourse.tile as tile
    from concourse import bacc
    from contextlib import ExitStack

    nc = bacc.Bacc()
    q = nc.declare_dram_parameter("q", [SEQ, QH * D], mybir.dt.float32, isOutput=False)
    k = nc.declare_dram_parameter("k", [SEQ, D], mybir.dt.float32, isOutput=False)
    v = nc.declare_dram_parameter("v", [SEQ, D], mybir.dt.float32, isOutput=False)
    out = nc.declare_dram_parameter("out", [SEQ, QH * D], mybir.dt.float32, isOutput=True)

    with tile.TileContext(nc) as tc:
        with ExitStack() as ctx:
            _emit(ctx, tc, q[:], k[:], v[:], out[:])
    nc.compile()
    return nc


def _get_nc():
    global _NC
    if _NC is None:
        _NC = _build()
    return _NC


def _ensure_ntff_hook():
    """The agent image's antenv lacks axon_hooks; shim it so trace=True works."""
    import sys
    import types

    if "antenv.axon_hooks" in sys.modules:
        return
    try:
        import antenv
        from trn_agent_boot.trn_boot import _ntff_profile_via_ctypes
    except ImportError:
        return
    mod = types.ModuleType("antenv.axon_hooks")
    hook = [None]
    mod.set_axon_ntff_profile_hook = lambda h: hook.__setitem__(0, h)
    mod.get_axon_ntff_profile_hook = lambda: hook[0]
    sys.modules["antenv.axon_hooks"] = mod
    antenv.axon_hooks = mod
    mod.set_axon_ntff_profile_hook(_ntff_profile_via_ctypes("/opt/axon/libaxon_pjrt.so"))


def _run(q, k, v, trace=False):
    from concourse.bass_utils import run_bass_kernel_spmd

    if trace:
        _ensure_ntff_hook()
    nc = _get_nc()
    in_maps = []
    for i in range(N_CORES):
        in_maps.append(
            {
                "q": np.ascontiguousarray(q[:, i * QH * D:(i + 1) * QH * D]).astype(np.float32, copy=False),
                "k": np.ascontiguousarray(k[:, i * D:(i + 1) * D]).astype(np.float32, copy=False),
                "v": np.ascontiguousarray(v[:, i * D:(i + 1) * D]).astype(np.float32, copy=False),
            }
        )
    res = run_bass_kernel_spmd(nc, in_maps, core_ids=list(range(N_CORES)), trace=trace)
    full = np.concatenate([res.results[i]["out"] for i in range(N_CORES)], axis=1)
    return full.astype(np.float32, copy=False), res


def kernel(q, k, v):
    out, _ = _run(q, k, v, trace=False)
    return out


# revision 26
# speedup vs baseline: 1.0098x; 1.0098x over previous
"""GQA causal attention (S=2048, H=32, KVH=8, D=128) on 8 TRN2 NeuronCores.

Sharding: tensor-parallel over heads. Core i computes query heads
[4i, 4i+4) against KV head i (GQA group size 32/8 = 4). No collectives:
the host slices the inputs per core and concatenates the outputs.

Per-core algorithm (seq=2048, d=128, 4 q-heads, 1 kv-head, causal):
  - K^T and per-head Q^T staged in SBUF as [d=128, seq] bf16
    (PE transposes via identity matmul; fp32 DMA-transpose is unsupported
    and the xbar ucode transpose costs ~1.3us of SP-engine time per call).
  - V staged naturally as [128, 16, 129] bf16 tiles with a ones column
    appended, so the PV matmul also produces the softmax denominator.
  - For each head, for each key-tile kt (128 keys):
      S^T[kt]  = (K^T tile).T @ Q^T          -> PSUM [128, qspan] fp32,
                 exact-causal: only q >= kt*128 is computed
      P^T[kt]  = exp(SCALE * S^T[kt])        -> SBUF bf16 (wide ACTIVATEs;
                 scores are O(1) so no max-subtraction is needed)
      the diagonal 128-column block is masked with gpsimd.affine_select
  - For each query-tile qt: acc[qt] = sum_kt (P^T tile).T @ [V | 1]
      accumulated in PSUM over kt; column 128 is the denominator.
      DVE reciprocal + tensor_scalar_mul normalizes into a per-q-block
      staging buffer; one batched DMA per 512 rows stores the output.
"""

import numpy as np

SEQ = 2048
D = 128
QH = 4  # query heads per core
N_CORES = 8
SCALE = 0.08838834764831845  # 1/sqrt(128)
NT = SEQ // 128  # 16 tiles of 128 along seq

_NC = None


def _emit(ctx, tc, q, k, v, out):
    import concourse.mybir as mybir
    from concourse import masks

    nc = tc.nc
    f32 = mybir.dt.float32
    bf16 = mybir.dt.bfloat16
    Exp = mybir.ActivationFunctionType.Exp

    # Every DMA destination gets a dedicated (never-recycled) buffer: a
    # reused slot would add extra semaphore waits on the HWDGE DMA.
    singles = ctx.enter_context(tc.tile_pool(name="singles", bufs=1))
    qpool = ctx.enter_context(tc.tile_pool(name="qpool", bufs=2))
    ppool = ctx.enter_context(tc.tile_pool(name="ppool", bufs=2))
    opool = ctx.enter_context(tc.tile_pool(name="opool", bufs=3))
    # PSUM budget (8 banks): scores 2x2 + out-acc 2x1 + transposes 2x1.
    psum_s = ctx.enter_context(tc.tile_pool(name="psum_s", bufs=2, space="PSUM"))
    psum_o = ctx.enter_context(tc.tile_pool(name="psum_o", bufs=2, space="PSUM"))
    psum_t = ctx.enter_context(tc.tile_pool(name="psum_t", bufs=2, space="PSUM"))

    # ---- PE warmup: dense dummy matmuls while the DMA prep runs, so the
    # HAM clock-gate reaches 2.4 GHz by the time real PE work arrives.
    warm_src = singles.tile([128, 512], bf16, tag="warm_src")
    nc.vector.memset(warm_src[:], 0.0)
    warm_ps = psum_o.tile([128, 512], f32, tag="o")
    for _ in range(12):
        nc.tensor.matmul(
            warm_ps[:], lhsT=warm_src[:, 0:128], rhs=warm_src[:], start=True, stop=True
        )

    ident = singles.tile([128, 128], bf16)
    masks.make_identity(nc, ident[:])
    keep = singles.tile([128, 128], bf16)
    masks.make_upper_triangular(nc, keep[:], val=1.0, diag=True)

    kT = singles.tile([128, SEQ], bf16)
    knat = singles.tile([128, NT, 128], f32, tag="knat")
    knat_bf = singles.tile([128, NT, 128], bf16, tag="knat_bf")
    kr = k.rearrange("(t p) d -> p t d", p=128)

    def kchunk(c, copy_eng):
        """Load + cast + PE-transpose one 4-tile chunk of K into kT."""
        cs = slice(c * 4, (c + 1) * 4)
        nc.sync.dma_start(out=knat[:, cs, :], in_=kr[:, cs, :])
        nc.vector.tensor_copy(knat_bf[:, cs, :], knat[:, cs, :])
        for t in range(c * 4, (c + 1) * 4):
            pst = psum_t.tile([128, 128], bf16, tag="tp")
            nc.tensor.transpose(pst[:], knat_bf[:, t, :], ident[:])
            copy_eng(kT[:, t * 128:(t + 1) * 128], pst[:])

    def qprep_alloc(h):
        qnat = singles.tile([128, NT, 128], f32, tag=f"qnat{h}")
        qnat_bf = singles.tile([128, NT, 128], bf16, tag=f"qnat_bf{h}")
        qT = qpool.tile([128, SEQ], bf16, tag="qT")
        return qnat, qnat_bf, qT

    def qprep_chunk(h, st, c):
        """Load + cast + PE-transpose one 4-tile chunk of head h's Q."""
        qnat, qnat_bf, qT = st
        qrh = q[:, h * D:(h + 1) * D].rearrange("(t p) d -> p t d", p=128)
        cs = slice(c * 4, (c + 1) * 4)
        nc.sync.dma_start(out=qnat[:, cs, :], in_=qrh[:, cs, :])
        nc.vector.tensor_copy(qnat_bf[:, cs, :], qnat[:, cs, :])
        for t in range(c * 4, (c + 1) * 4):
            pst = psum_t.tile([128, 128], bf16, tag="tp")
            nc.tensor.transpose(pst[:], qnat_bf[:, t, :], ident[:])
            nc.vector.tensor_copy(qT[:, t * 128:(t + 1) * 128], pst[:])

    def emit_qprep(h):
        st = qprep_alloc(h)
        for c in range(4):
            qprep_chunk(h, st, c)
        return st[2]

    # ---- Prep, ordered for shortest path to the first QK matmul: K chunk 0
    # and head-0 Q chunks 0-1 only; the rest is emitted inside the head-0
    # kt loop so the PE's in-order stream reaches QK(kt=0) early.
    kchunk(0, nc.vector.tensor_copy)
    q0st = qprep_alloc(0)
    qprep_chunk(0, q0st, 0)
    qprep_chunk(0, q0st, 1)
    qT = q0st[2]

    # ---- V: natural [128, t, d] bf16 + ones column for the denominator
    vp = singles.tile([128, NT, D + 1], bf16)
    vnat = singles.tile([128, NT, 128], f32, tag="vnat")

    def vprep():
        nc.sync.dma_start(out=vnat[:], in_=v.rearrange("(t p) d -> p t d", p=128))
        nc.vector.tensor_copy(vp[:, :, 0:D], vnat[:])
        nc.vector.memset(vp[:, :, D:D + 1], 1.0)

    def emit_pv(h, qt, pT, vp, osb, ops_tri):
        """O[qt] = sum_k2 pT[k2][:, qt-slice].T @ [V|1], then normalize."""
        ops = ops_tri[:, qt % 3, :]
        for k2 in range(qt + 1):
            nc.tensor.matmul(
                ops,
                lhsT=pT[k2][:, (qt - k2) * 128:(qt - k2) * 128 + 128],
                rhs=vp[:, k2, :],
                start=(k2 == 0),
                stop=(k2 == qt),
            )
        rec = opool.tile([128, 1], f32, tag="rec")
        nc.vector.reciprocal(rec[:], ops[:, D:D + 1])
        nc.vector.tensor_scalar_mul(osb[:, qt % 2, :], ops[:, 0:D], rec[:])
        if qt % 2 == 1:
            qb = qt // 2
            nc.sync.dma_start(
                out=out[qb * 256:(qb + 1) * 256, h * D:(h + 1) * D].rearrange(
                    "(j p) d -> p j d", p=128
                ),
                in_=osb[:],
            )
    def emit_qk_exp(qT, kt, pT_kt, off, cw):
        """One exact-causal S^T chunk ([k0+off, k0+off+cw)) + its exp."""
        k0 = kt * 128
        pw = ((cw + 511) // 512) * 512
        sp = psum_s.tile([128, pw], f32, tag="s")
        m = 0
        while m < cw:
            w = min(512, cw - m)
            nc.tensor.matmul(
                sp[:, m:m + w],
                lhsT=kT[:, k0:k0 + 128],
                rhs=qT[:, k0 + off + m:k0 + off + m + w],
                start=True,
                stop=True,
            )
            m += w
        nc.scalar.activation(pT_kt[:, off:off + cw], sp[:, 0:cw], Exp, scale=SCALE)

    # Pending-PV queue: PV work is emitted two QK steps behind, flowing
    # across head boundaries so neither the PE nor ScalarE sees a bubble
    # between heads.
    pvq = []
    pv_state = {}

    def pop_pv():
        h2, qt2, pT2 = pvq.pop(0)
        st = pv_state.setdefault(h2, {})
        if qt2 % 2 == 0:
            osb = opool.tile([128, 2, D], f32, tag="osb")
            st["osb"] = osb
        if qt2 % 3 == 0:
            ops = psum_o.tile([128, 3, D + 1], f32, tag="o")
            st["ops"] = ops
        emit_pv(h2, qt2, pT2, vp, st["osb"], st["ops"])

    for h in range(QH):
        qT_next = None
        pT = []
        for kt in range(NT):
            k0 = kt * 128
            span = SEQ - k0
            pT_kt = ppool.tile([128, span], bf16, tag=f"pT{kt}")
            # Exact-causal S^T in left-aligned PSUM chunks of <=1024
            # (2 banks), one wide exp each. On head 0's first key-tile the
            # remaining prep is interleaved between chunks so the PE
            # reaches the first QK matmul as early as possible.
            off = 0
            while off < span:
                cw = min(1024, span - off)
                emit_qk_exp(qT, kt, pT_kt, off, cw)
                off += cw
                if h == 0 and kt == 0 and off == 1024:
                    qprep_chunk(0, q0st, 2)
                    qprep_chunk(0, q0st, 3)
            # causal mask on the diagonal 128-col block: keep where q >= k
            nc.vector.tensor_mul(pT_kt[:, 0:128], pT_kt[:, 0:128], keep[:])
            pT.append(pT_kt)
            if h == 0 and kt < 3:
                kchunk(kt + 1, nc.vector.tensor_copy)
                if kt == 0:
                    vprep()
            pvq.append((h, kt, pT))
            while len(pvq) > 3:
                pop_pv()
            # prefetch the next head's Q transposes into the PE stream,
            # one chunk per kt step to avoid a transpose burst
            if h + 1 < QH:
                if kt == 10:
                    qst_next = qprep_alloc(h + 1)
                    qT_next = qst_next[2]
                if 10 <= kt <= 13:
                    qprep_chunk(h + 1, qst_next, kt - 10)
        if qT_next is not None:
            qT = qT_next
    while pvq:
        pop_pv()


def _build():
    import concourse.mybir as mybir
    import concourse.tile as tile
    from concourse import bacc
    from contextlib import ExitStack

    nc = bacc.Bacc()
    q = nc.declare_dram_parameter("q", [SEQ, QH * D], mybir.dt.float32, isOutput=False)
    k = nc.declare_dram_parameter("k", [SEQ, D], mybir.dt.float32, isOutput=False)
    v = nc.declare_dram_parameter("v", [SEQ, D], mybir.dt.float32, isOutput=False)
    out = nc.declare_dram_parameter("out", [SEQ, QH * D], mybir.dt.float32, isOutput=True)

    with tile.TileContext(nc) as tc:
        with ExitStack() as ctx:
            _emit(ctx, tc, q[:], k[:], v[:], out[:])
    nc.compile()
    return nc


def _get_nc():
    global _NC
    if _NC is None:
        _NC = _build()
    return _NC


def _ensure_ntff_hook():
    """The agent image's antenv lacks axon_hooks; shim it so trace=True works."""
    import sys
    import types

    if "antenv.axon_hooks" in sys.modules:
        return
    try:
        import antenv
        from trn_agent_boot.trn_boot import _ntff_profile_via_ctypes
    except ImportError:
        return
    mod = types.ModuleType("antenv.axon_hooks")
    hook = [None]
    mod.set_axon_ntff_profile_hook = lambda h: hook.__setitem__(0, h)
    mod.get_axon_ntff_profile_hook = lambda: hook[0]
    sys.modules["antenv.axon_hooks"] = mod
    antenv.axon_hooks = mod
    mod.set_axon_ntff_profile_hook(_ntff_profile_via_ctypes("/opt/axon/libaxon_pjrt.so"))


def _run(q, k, v, trace=False):
    from concourse.bass_utils import run_bass_kernel_spmd

    if trace:
        _ensure_ntff_hook()
    nc = _get_nc()
    in_maps = []
    for i in range(N_CORES):
        in_maps.append(
            {
                "q": np.ascontiguousarray(q[:, i * QH * D:(i + 1) * QH * D]).astype(np.float32, copy=False),
                "k": np.ascontiguousarray(k[:, i * D:(i + 1) * D]).astype(np.float32, copy=False),
                "v": np.ascontiguousarray(v[:, i * D:(i + 1) * D]).astype(np.float32, copy=False),
            }
        )
    res = run_bass_kernel_spmd(nc, in_maps, core_ids=list(range(N_CORES)), trace=trace)
    full = np.concatenate([res.results[i]["out"] for i in range(N_CORES)], axis=1)
    return full.astype(np.float32, copy=False), res


def kernel(q, k, v):
    out, _ = _run(q, k, v, trace=False)
    return out
